# revision 1
# baseline (speedup 1.0000x reference)
"""Trainium2 Bass kernel for nn_NeuralRenderer.

Renders B=16 images of 256x256 pixels from C=64 circles each:
  depth(b,p) = min_c [ dist(p, center_bc) < R_c ?  D_bc - sqrt(R_c^2 - dist^2) : Dfar ]

Sharding: data-parallel over batch. 8 cores x 2 batches each.

Per-core layout: image = [128 partitions x 512 free] (partition k holds two
image rows 2k, 2k+1). Loop over circles, accumulate a negated running max:
  acc = max_c (s_c - D_c),  out = -acc,  with s_c = sqrt(R^2 - d2) for inside
pixels, and s_c overwritten with -712 for outside pixels (so s - D < -512
never wins against the -Dfar init).

Exactness of the inside test: the reference computes `fl(sqrt(d2)) < R`.
We precompute on host Tm = the largest fp32 t with fl(sqrt(t)) < R, so the
test `d2 <= Tm` is bit-exact equivalent (sqrt is monotone, correctly
rounded). The epsilon (+1e-12 per coordinate) in the reference is absorbed
by fp32 rounding everywhere it could affect the output.

All per-core inputs are packed into ONE dram tensor -> one DMA -> one
semaphore (TensorScalar ISA slots allow few waits).

Engine assignment per circle-group (4 circles, free=512 per circle):
  DVE : dx,dy (tensor_scalar 2x), d2 = sx+sy (TT), clamp (TS 2x),
        copy_predicated (outside -> -712), fused (s-D) max acc (STT)
  ACT : squares (batched [128,4096], bias=0), sqrt (batched, scale=-1)
  GPS : outside mask (tensor_scalar is_gt, uint8)
"""

import numpy as np

LAST_EXEC_NS = None

B, C, DIM = 16, 64, 256
P = DIM * DIM
N_CORES = 8
B_PER_CORE = B // N_CORES          # 2
PARTS = 128
FREE = P // PARTS                  # 512
GROUP = 4                          # circles per ACT batch
OUTSIDE_S = -712.0                 # sentinel: s-D <= -712 < -Dfar always loses

# packed input column offsets
_XT0 = 0
_YT0 = FREE
_NU0 = 2 * FREE                    # + 64*b
_NV0 = _NU0 + C * B_PER_CORE
_DD0 = _NV0 + C * B_PER_CORE
_TM0 = _DD0 + C * B_PER_CORE
_INW = _TM0 + C * B_PER_CORE       # 1536


def _compute_Tm(R):
    """Largest fp32 t with fl(sqrt(t)) < R (host, exact)."""
    R = np.float32(R)
    t = np.float32(R) * np.float32(R)
    while not (np.sqrt(t, dtype=np.float32) < R):
        t = np.nextafter(t, np.float32(0), dtype=np.float32)
    while True:
        t_next = np.nextafter(t, np.float32(np.inf), dtype=np.float32)
        if np.sqrt(t_next, dtype=np.float32) < R:
            t = t_next
        else:
            break
    return t


def _build_bass(dfar):
    import concourse.mybir as mybir
    from concourse.bacc import Bacc
    from concourse.mybir import AluOpType
    from concourse.tile import TileContext

    nc = Bacc(trn_type="TRN2")
    f32 = mybir.dt.float32

    inp_d = nc.dram_tensor("inp", [PARTS, _INW], f32, kind="ExternalInput")
    out_d = nc.dram_tensor("out", [B_PER_CORE, PARTS, FREE], f32,
                           kind="ExternalOutput")

    GF = GROUP * FREE  # 2048

    with TileContext(nc) as tc:
        with tc.tile_pool(name="static", bufs=1) as sp, \
             tc.tile_pool(name="work", bufs=2) as wp, \
             tc.tile_pool(name="accp", bufs=1) as ap:
            inp = sp.tile([PARTS, _INW], f32)
            nc.sync.dma_start(inp[:], inp_d[:])
            xt = inp[:, _XT0:_XT0 + FREE]
            yt = inp[:, _YT0:_YT0 + FREE]

            c712 = sp.tile([PARTS, GF], f32, name="c712", tag="c712")
            nc.vector.memset(c712[:], OUTSIDE_S)

            # prime GPSIMD's view of the input DMA semaphore: TS-struct
            # instructions only fit one sync wait, so the per-iteration mask
            # op must only ever need the DVE wait.
            gprime = sp.tile([PARTS, 1], f32, name="gprime", tag="gprime")
            nc.gpsimd.tensor_copy(gprime[:], inp[:, _TM0:_TM0 + 1])

            accs = []
            for b in range(B_PER_CORE):
                acc = ap.tile([PARTS, FREE], f32, name=f"acc{b}", tag=f"acc{b}")
                nc.vector.memset(acc[:], -dfar)
                accs.append(acc)

            for b in range(B_PER_CORE):
                nu = inp[:, _NU0 + C * b:_NU0 + C * (b + 1)]
                nv = inp[:, _NV0 + C * b:_NV0 + C * (b + 1)]
                dd = inp[:, _DD0 + C * b:_DD0 + C * (b + 1)]
                tm = inp[:, _TM0 + C * b:_TM0 + C * (b + 1)]
                acc = accs[b]
                for g in range(C // GROUP):
                    dxy = wp.tile([PARTS, 2 * GF], f32, tag="dxy")
                    sq = wp.tile([PARTS, 2 * GF], f32, tag="sq")
                    d2 = wp.tile([PARTS, GF], f32, tag="d2")
                    w = wp.tile([PARTS, GF], f32, tag="w")
                    s = wp.tile([PARTS, GF], f32, tag="s")
                    mk = wp.tile([PARTS, GF], mybir.dt.uint8, tag="mk")
                    for k in range(GROUP):
                        c = g * GROUP + k
                        ks = slice(k * FREE, (k + 1) * FREE)
                        ks2 = slice((GROUP + k) * FREE, (GROUP + k + 1) * FREE)
                        # dx = x - u ; dy = y - v   (TS, 2x mode)
                        nc.vector.tensor_scalar(
                            dxy[:, ks], xt, nu[:, c:c + 1], None,
                            AluOpType.subtract)
                        nc.vector.tensor_scalar(
                            dxy[:, ks2], yt, nv[:, c:c + 1], None,
                            AluOpType.subtract)
                    # squares, batched (both dx and dy blocks): [128, 4096]
                    nc.scalar.activation(
                        sq[:], dxy[:], mybir.ActivationFunctionType.Square)
                    for k in range(GROUP):
                        c = g * GROUP + k
                        ks = slice(k * FREE, (k + 1) * FREE)
                        ks2 = slice((GROUP + k) * FREE, (GROUP + k + 1) * FREE)
                        # d2 = sx + sy
                        nc.vector.tensor_tensor(
                            d2[:, ks], sq[:, ks], sq[:, ks2], AluOpType.add)
                        # outside mask: d2 > Tm  (gpsimd)
                        nc.gpsimd.tensor_scalar(
                            mk[:, ks], d2[:, ks], tm[:, c:c + 1], None,
                            AluOpType.is_gt)
                        # w = min(d2, Tm) - Tm  (<= 0), fused TS
                        nc.vector.tensor_scalar(
                            w[:, ks], d2[:, ks], tm[:, c:c + 1], tm[:, c:c + 1],
                            AluOpType.min, AluOpType.subtract)
                    # s = sqrt(-w), batched [128, 2048]
                    nc.scalar.activation(
                        s[:], w[:], mybir.ActivationFunctionType.Sqrt,
                        bias=0.0, scale=-1.0)
                    # absorb the GPS wait on DVE (1 sync-wait slot per
                    # instruction): observe the last mask write, so
                    # copy_predicated below only waits on ACT.
                    mkd = wp.tile([PARTS, 1], mybir.dt.uint8, tag="mkd")
                    nc.vector.tensor_copy(mkd[:], mk[:, GF - 1:GF])
                    # outside: s <- -712
                    nc.vector.copy_predicated(s[:], mk[:], c712[:])
                    for k in range(GROUP):
                        c = g * GROUP + k
                        ks = slice(k * FREE, (k + 1) * FREE)
                        # acc = max(acc, s - D)   (fused STT)
                        nc.vector.scalar_tensor_tensor(
                            acc[:], s[:, ks], dd[:, c:c + 1], acc[:],
                            AluOpType.subtract, AluOpType.max)

            for b in range(B_PER_CORE):
                out_t = wp.tile([PARTS, FREE], f32, tag="out_t")
                # out = -acc
                nc.scalar.activation(
                    out_t[:], accs[b][:], mybir.ActivationFunctionType.Copy,
                    bias=0.0, scale=-1.0)
                nc.sync.dma_start(out_d[b], out_t[:])

    # bacc legalization: splits multi-waits into EventSemaphore instructions
    # (walrus codegen fits only one sync wait per instruction), fuses nops,
    # allocates registers.
    nc.compile()
    return nc


def kernel(uvd, UV, Radius, Dfar):
    import concourse.bass_utils as bass_utils

    uvd = np.asarray(uvd, dtype=np.float32)
    UV = np.asarray(UV, dtype=np.float32)
    Radius = np.asarray(Radius, dtype=np.float32)
    dfar = float(np.asarray(Dfar))

    xs = UV[0, 0, :].astype(np.float32).reshape(PARTS, FREE)
    ys = UV[0, 1, :].astype(np.float32).reshape(PARTS, FREE)

    Tm = np.array([_compute_Tm(Radius[c, 0]) for c in range(C)],
                  dtype=np.float32)                       # (C,)

    u = uvd[:, :, 0]                                      # (B,C)
    v = uvd[:, :, 1]
    D = uvd[:, :, 2]

    nc = _build_bass(dfar)

    in_maps = []
    for core in range(N_CORES):
        A = np.zeros((PARTS, _INW), dtype=np.float32)
        A[:, _XT0:_XT0 + FREE] = xs
        A[:, _YT0:_YT0 + FREE] = ys
        for b in range(B_PER_CORE):
            gb = core * B_PER_CORE + b
            A[:, _NU0 + C * b:_NU0 + C * (b + 1)] = u[gb][None, :]
            A[:, _NV0 + C * b:_NV0 + C * (b + 1)] = v[gb][None, :]
            A[:, _DD0 + C * b:_DD0 + C * (b + 1)] = D[gb][None, :]
            A[:, _TM0 + C * b:_TM0 + C * (b + 1)] = Tm[None, :]
        in_maps.append({"inp": A})

    res = bass_utils.run_bass_kernel_spmd(
        nc, in_maps, core_ids=list(range(N_CORES)))
    global LAST_EXEC_NS
    LAST_EXEC_NS = res.exec_time_ns

    out = np.empty((B, P), dtype=np.float32)
    for core in range(N_CORES):
        o = res.results[core]["out"]                      # (B_PER_CORE,128,512)
        out[core * B_PER_CORE:(core + 1) * B_PER_CORE] = o.reshape(
            B_PER_CORE, P)
    return out.reshape(B, 1, DIM, DIM)

